# revision 3
# baseline (speedup 1.0000x reference)
"""Trainium2 Bass kernel for nn_Decoder_70033736728716.

GRU decoder: latent -> MLP -> 39 sequential GRU steps with autoregressive
logits feedback -> y [B, 40, 21].

Strategy:
  - Pure data parallel over 8 cores (batch 16384 -> 2048 rows/core).
  - All compute in "transposed" orientation: state h kept as h.T
    [H=256 rows -> 2 chunks of 128 partitions, batch on the free dim],
    so the recurrent matmuls need no per-step transposes.
  - The autoregressive input x_t = pred_{t-1} = h_t @ Wd.T + bd is linear
    in h_t, so it is folded into the recurrent weights:
        Wbig = [ (Wd.T@Wih.T + Whh.T)[:, :2H] |  (Wd.T@Wih.T)[:, 2H:] | Whh.T[:, 2H:] ]
    giving ONE K=256 x N=1024 matmul per step (+ a tiny Wd matmul for the
    pred output itself).
  - f32r (TF32-like, ~1e-4 rel err) matmuls: 4x faster than fp32 on the PE.
  - Per-core batch is processed as 8 interleaved tiles of 256 columns so
    PE / ACT / DVE / DMA pipeline across tiles within each time step.
"""

import numpy as np

B, L, H, A = 16384, 128, 256, 21
T = 40  # output sequence length; T-1 = 39 GRU steps
NCORES = 8
BC = B // NCORES  # 2048 rows per core
NB = 256  # batch tile (free dim) size
NT = BC // NB  # 8 tiles per core
STEPS = T - 1  # 39


def _build(nc, tc, steps=STEPS, nt=NT):
    import concourse.bass as bass  # noqa
    import concourse.mybir as mybir

    F32 = mybir.dt.float32
    F32R = mybir.dt.float32r
    AF = mybir.ActivationFunctionType
    ALU = mybir.AluOpType

    XT = nc.dram_tensor("XT", [128, nt * NB], F32R, kind="ExternalInput")
    W1T = nc.dram_tensor("W1T", [128, 256], F32R, kind="ExternalInput")
    W2B = nc.dram_tensor("W2B", [2, 2, 128, 128], F32R, kind="ExternalInput")
    WB = nc.dram_tensor("WB", [2, 8, 128, 128], F32R, kind="ExternalInput")
    W0B = nc.dram_tensor("W0B", [2, 6, 128, 128], F32R, kind="ExternalInput")
    WDB = nc.dram_tensor("WDB", [2, 128, 21], F32R, kind="ExternalInput")
    # bias columns: brz0(0:4) brz(4:8) bin0(8:10) bin(10:12) bhn(12:14)
    #               b1(14:16) b2(16:18)
    BIAS = nc.dram_tensor("BIAS", [128, 18], F32, kind="ExternalInput")
    YT = nc.dram_tensor("YT", [steps, 21, nt * NB], F32, kind="ExternalOutput")

    from contextlib import ExitStack

    ctx = ExitStack()
    wts = ctx.enter_context(tc.tile_pool(name="wts", bufs=1))
    sb = ctx.enter_context(tc.tile_pool(name="sb", bufs=1))
    ps = ctx.enter_context(tc.tile_pool(name="ps", bufs=1, space="PSUM"))

    wb = wts.tile([128, 2, 8, 128], F32R, tag="wb")
    nc.sync.dma_start(wb[:], WB[:].rearrange("k c p m -> p k c m"))
    w0 = wts.tile([128, 2, 6, 128], F32R, tag="w0")
    nc.sync.dma_start(w0[:], W0B[:].rearrange("k c p m -> p k c m"))
    wd = wts.tile([128, 2, 21], F32R, tag="wd")
    nc.sync.dma_start(wd[:], WDB[:].rearrange("k p m -> p k m"))
    w1 = wts.tile([128, 2, 128], F32R, tag="w1")
    nc.sync.dma_start(w1[:], W1T[:].rearrange("p (c m) -> p c m", c=2))
    w2 = wts.tile([128, 2, 2, 128], F32R, tag="w2")
    nc.sync.dma_start(w2[:], W2B[:].rearrange("k c p m -> p k c m"))
    bias = wts.tile([128, 18], F32, tag="bias")
    nc.sync.dma_start(bias[:], BIAS[:])

    def bcol(c):
        return bias[:, c : c + 1]

    # ---- MLP prologue: h0.T = W2 @ relu(W1 @ x.T + b1) + b2, per tile ----
    hcur = []
    for i in range(nt):
        xt = sb.tile([128, NB], F32R, tag="xt", bufs=3, name=f"xt{i}")
        nc.sync.dma_start(xt[:], XT[:, i * NB : (i + 1) * NB])
        ps1 = ps.tile([128, 2, NB], F32, tag="rz", bufs=2, name=f"ps1_{i}")
        for c in range(2):
            nc.tensor.matmul(ps1[:, c, :], w1[:, c, :], xt[:])
        u1 = sb.tile([128, 2, NB], F32R, tag="u1", bufs=2, name=f"u1_{i}")
        for c in range(2):
            nc.vector.tensor_scalar(
                u1[:, c, :], ps1[:, c, :], bcol(14 + c), 0.0, op0=ALU.add, op1=ALU.max
            )
        ps2 = ps.tile([128, 2, NB], F32, tag="nn", bufs=2, name=f"ps2_{i}")
        for c in range(2):
            for k in range(2):
                nc.tensor.matmul(
                    ps2[:, c, :], w2[:, k, c, :], u1[:, k, :],
                    start=(k == 0), stop=(k == 1),
                )
        h0 = sb.tile([128, 2, NB], F32R, tag=f"h{i}", bufs=2, name=f"h0_{i}")
        for c in range(2):
            nc.vector.tensor_scalar_add(h0[:, c, :], ps2[:, c, :], bcol(16 + c))
        hcur.append(h0)

    # ---- GRU steps ----
    for t in range(steps):
        first = t == 0
        for i in range(nt):
            h = hcur[i]
            rz_ps = ps.tile([128, 4, NB], F32, tag="rz", bufs=2, name=f"rz_{t}_{i}")
            nn_ps = ps.tile([128, 4, NB], F32, tag="nn", bufs=2, name=f"nn_{t}_{i}")
            if first:
                for c in range(4):
                    for k in range(2):
                        nc.tensor.matmul(
                            rz_ps[:, c, :], w0[:, k, c, :], h[:, k, :],
                            start=(k == 0), stop=(k == 1),
                        )
                for c in range(2):
                    for k in range(2):
                        nc.tensor.matmul(
                            nn_ps[:, 2 + c, :], w0[:, k, 4 + c, :], h[:, k, :],
                            start=(k == 0), stop=(k == 1),
                        )
            else:
                for c in range(4):
                    for k in range(2):
                        nc.tensor.matmul(
                            rz_ps[:, c, :], wb[:, k, c, :], h[:, k, :],
                            start=(k == 0), stop=(k == 1),
                        )
                for c in range(4):
                    for k in range(2):
                        nc.tensor.matmul(
                            nn_ps[:, c, :], wb[:, k, 4 + c, :], h[:, k, :],
                            start=(k == 0), stop=(k == 1),
                        )
                # pred_{t-1} = h_t @ Wd.T  (bd added on host)
                pred_ps = ps.tile([21, NB], F32, tag="rz", bufs=2, name=f"pp_{t}_{i}")
                for k in range(2):
                    nc.tensor.matmul(
                        pred_ps[:], wd[:, k, :], h[:, k, :],
                        start=(k == 0), stop=(k == 1),
                    )
                pred_sb = sb.tile([21, NB], F32, tag="predsb", bufs=3, name=f"po_{t}_{i}")
                nc.vector.tensor_copy(pred_sb[:], pred_ps[:])
                nc.sync.dma_start(YT[t - 1, :, i * NB : (i + 1) * NB], pred_sb[:])

            rzb = 0 if first else 4
            rz = sb.tile([128, 4, NB], F32, tag="rzsb", bufs=2, name=f"rs_{t}_{i}")
            for c in range(4):
                nc.scalar.activation(
                    rz[:, c, :], rz_ps[:, c, :], AF.Sigmoid, bias=bcol(rzb + c)
                )
            t1 = sb.tile([128, 2, NB], F32, tag="t1", bufs=2, name=f"t1_{t}_{i}")
            # t1 = (hn + bhn) * r
            for c in range(2):
                nc.vector.scalar_tensor_tensor(
                    t1[:, c, :], nn_ps[:, 2 + c, :], bcol(12 + c), rz[:, c, :],
                    op0=ALU.add, op1=ALU.mult,
                )
            n_sb = sb.tile([128, 2, NB], F32, tag="nsb", bufs=2, name=f"ns_{t}_{i}")
            if first:
                # n = tanh(t1 + bin0)   (inn is constant at t=0)
                for c in range(2):
                    nc.scalar.activation(
                        n_sb[:, c, :], t1[:, c, :], AF.Tanh, bias=bcol(8 + c)
                    )
            else:
                # t1 += inn + bin ; n = tanh(t1)
                for c in range(2):
                    nc.vector.scalar_tensor_tensor(
                        t1[:, c, :], nn_ps[:, c, :], bcol(10 + c), t1[:, c, :],
                        op0=ALU.add, op1=ALU.add,
                    )
                nc.scalar.activation(n_sb[:], t1[:], AF.Tanh)
            # h_new = (h - n) * z + n
            u = sb.tile([128, 2, NB], F32, tag="u", bufs=2, name=f"u_{t}_{i}")
            nc.vector.tensor_sub(u[:], h[:].bitcast(F32), n_sb[:])
            nc.vector.tensor_mul(u[:], u[:], rz[:, 2:4, :])
            hnew = sb.tile([128, 2, NB], F32R, tag=f"h{i}", bufs=2, name=f"h_{t + 1}_{i}")
            nc.vector.tensor_add(hnew[:], u[:], n_sb[:])
            hcur[i] = hnew

    # ---- epilogue: pred_{steps-1} from final h ----
    for i in range(nt):
        h = hcur[i]
        pred_ps = ps.tile([21, NB], F32, tag="rz", bufs=2, name=f"pp_end_{i}")
        for k in range(2):
            nc.tensor.matmul(
                pred_ps[:], wd[:, k, :], h[:, k, :], start=(k == 0), stop=(k == 1)
            )
        pred_sb = sb.tile([21, NB], F32, tag="predsb", bufs=3, name=f"po_end_{i}")
        nc.vector.tensor_copy(pred_sb[:], pred_ps[:])
        nc.sync.dma_start(YT[steps - 1, :, i * NB : (i + 1) * NB], pred_sb[:])
    ctx.close()


def _prep_host(latent, W1, b1, W2, b2, Wih, Whh, bih, bhh, Wd, bd):
    f8 = np.float64
    WdWih = (Wd.T.astype(f8) @ Wih.T.astype(f8)).astype(np.float32)  # [256, 768]
    WhhT = np.ascontiguousarray(Whh.T)  # [256, 768]
    Wbig = np.concatenate(
        [
            WdWih[:, : 2 * H] + WhhT[:, : 2 * H],
            WdWih[:, 2 * H :],
            WhhT[:, 2 * H :],
        ],
        axis=1,
    )  # [256, 1024]
    start_row = np.full((A,), -32.0, np.float32)
    start_row[0] = 32.0
    gi0 = (start_row.astype(f8) @ Wih.T.astype(f8) + bih.astype(f8)).astype(np.float32)
    bd_wih = (bd.astype(f8) @ Wih.T.astype(f8)).astype(np.float32)

    brz0 = gi0[: 2 * H] + bhh[: 2 * H]
    brz = bd_wih[: 2 * H] + bih[: 2 * H] + bhh[: 2 * H]
    bin0 = gi0[2 * H :]
    binn = bd_wih[2 * H :] + bih[2 * H :]
    bhn = bhh[2 * H :]

    bias = np.zeros((128, 18), np.float32)
    bias[:, 0:4] = brz0.reshape(4, 128).T
    bias[:, 4:8] = brz.reshape(4, 128).T
    bias[:, 8:10] = bin0.reshape(2, 128).T
    bias[:, 10:12] = binn.reshape(2, 128).T
    bias[:, 12:14] = bhn.reshape(2, 128).T
    bias[:, 14:16] = b1.reshape(2, 128).T
    bias[:, 16:18] = b2.reshape(2, 128).T

    def blocks(m, ncol):  # [256, ncol*128] -> [2, ncol, 128, 128]
        return np.ascontiguousarray(
            m.reshape(2, 128, ncol, 128).transpose(0, 2, 1, 3)
        )

    WBnp = blocks(Wbig, 8)
    W0np = blocks(WhhT[:, : 6 * 128], 6)
    WDnp = np.ascontiguousarray(Wd.T.reshape(2, 128, 21))
    W1Tnp = np.ascontiguousarray(W1.T)  # [128, 256]
    W2Tnp = np.ascontiguousarray(W2.T)  # [256, 256]
    W2np = np.ascontiguousarray(
        W2Tnp.reshape(2, 128, 2, 128).transpose(0, 2, 1, 3)
    )
    shared = {
        "W1T": W1Tnp,
        "W2B": W2np,
        "WB": WBnp,
        "W0B": W0np,
        "WDB": WDnp,
        "BIAS": bias,
    }
    start = np.full((B, A), -32.0, np.float32)
    start[:, 0] = 32.0
    return shared, start


def kernel(latent, W1, b1, W2, b2, Wih, Whh, bih, bhh, Wd, bd):
    import concourse.tile as tile
    from concourse import bacc
    from concourse.bass_utils import run_bass_kernel_spmd

    nc = bacc.Bacc("TRN2", target_bir_lowering=False, debug=False, num_devices=NCORES)
    with tile.TileContext(nc) as tc:
        _build(nc, tc)
    nc.compile()

    shared, start = _prep_host(
        latent, W1, b1, W2, b2, Wih, Whh, bih, bhh, Wd, bd
    )
    in_maps = []
    for core in range(NCORES):
        rows = slice(core * BC, (core + 1) * BC)
        xt = np.ascontiguousarray(latent[rows].T)  # [128, 2048]
        in_maps.append({**shared, "XT": xt})

    res = run_bass_kernel_spmd(nc, in_maps, list(range(NCORES)))

    y = np.empty((B, T, A), np.float32)
    y[:, 0, :] = start
    for core in range(NCORES):
        rows = slice(core * BC, (core + 1) * BC)
        yt = res.results[core]["YT"]  # [39, 21, 2048]
        y[rows, 1:, :] = yt.transpose(2, 0, 1) + bd[None, None, :]
    return y
